# revision 6
# baseline (speedup 1.0000x reference)
"""Trainium2 Bass kernel for nn_Evolution (gated sigmoid recurrence).

Math per step t (reference):
    z    = concat([h, s_t], -1) @ w1                  # [N, D]
    pre  = z * thr_t + h * (1 - thr_t)
    cand = sigmoid(pre) * exp(-1/2)
    delta = mean(|cand - h|, -1, keepdims=True)
    g    = sigmoid(gl + ss * delta)
    h    = g * h + (1 - g) * cand                     # = h + (1-g)*(cand-h)

Outputs: states [T,N,D] (h after each step), h_final [N,D], diffs
states[1:]-states[:-1] [T-1,N,D].

Device mapping (8 cores, N=4096 sharded 512 rows/core; 4 independent
128-row lanes per core):
  - matmul in bf16: z_lane = sum_k lhsT(k-tile).T @ w1(k-tile) with
    lhsT k-tiles 0-3 = transposed h (maintained on device), 4-7 = host
    pre-transposed s_t. Accumulate fp32 in PSUM. Errors are crushed by
    the thr*z saturation structure (z ~ 110..380, sigmoid 97% saturated;
    non-saturated rows have thr ~ 1/z so err_pre ~ err_z/z ~ 1e-5).
  - pre is computed as thr*(z + r*h) with r = (1-thr)/thr folded into a
    per-partition scalar (host-precomputed), which lets one fused
    scalar_tensor_tensor produce (z + r*h) and the activation's scale
    input apply thr while evaluating sigmoid.
  - 1-g = sigmoid(-gl - ss*delta) directly (per-partition scalar).
  - diffs come for free: diffs[t-1] = h_new - h = (1-g)*(cand-h) = gd.
  - h is re-transposed each step via 4 PE transposes (fp32) and cast to
    bf16 in the PSUM->SBUF copy.
"""

import math
from contextlib import ExitStack

import ml_dtypes
import numpy as np

import concourse.bass as bass
import concourse.mybir as mybir
import concourse.tile as tile


def split_excess_syncs(nc) -> int:
    """Split >1 sync waits/updates per instruction onto adjacent NoOps.

    The walrus in this environment lowers at most one sync wait command
    per instruction (CTRL formats); Tile's kernel-tail drain can carry
    more, failing codegen with "Too many sync wait commands". Moving the
    overflow onto NoOps just before (waits) / after (updates) on the
    same engine is semantically equivalent: per-engine programs execute
    in order.
    """
    MAXW = 1
    MAXU = 1
    nfix = 0
    for f in nc.m.functions:
        for bb in f.blocks:
            snapshot = list(bb.instructions)
            inserts = []  # (index, new_inst) — insert BEFORE index
            for idx, inst in enumerate(snapshot):
                si = inst.sync_info
                if si is None:
                    continue
                waits = list(si.on_wait)
                updates = list(si.on_update)
                if len(waits) <= MAXW and len(updates) <= MAXU:
                    continue
                pre = []
                rest_w = list(waits)
                while len(rest_w) > MAXW:
                    chunk, rest_w = rest_w[:MAXW], rest_w[MAXW:]
                    n = mybir.InstNoOp(
                        name=f"{inst.name}-wsp{len(pre)}", engine=inst.engine
                    )
                    n.sync_info = mybir.SyncInfo(on_wait=chunk, on_update=[])
                    pre.append(n)
                keep_u, rest_u = updates[:MAXU], updates[MAXU:]
                post = []
                while rest_u:
                    chunk, rest_u = rest_u[:MAXU], rest_u[MAXU:]
                    n = mybir.InstNoOp(
                        name=f"{inst.name}-usp{len(post)}", engine=inst.engine
                    )
                    n.sync_info = mybir.SyncInfo(on_wait=[], on_update=chunk)
                    post.append(n)
                inst.sync_info = mybir.SyncInfo(on_wait=rest_w, on_update=keep_u)
                for n in pre:
                    inserts.append((idx, n))
                for n in post:
                    inserts.append((idx + 1, n))
                nfix += 1
            live = bb.instructions
            for index, n in sorted(inserts, key=lambda kv: kv[0], reverse=True):
                live.insert(index, n)
    return nfix

T, N, D = 32, 4096, 512
NCORES = 8
NLOC = N // NCORES  # 512 rows per core
P = 128
LANES = NLOC // P  # 4
KT = (2 * D) // P  # 8 contraction k-tiles
MT = D // P  # 4 m-tiles (columns of h)
EXPH = math.exp(-0.5)

F32 = mybir.dt.float32
BF16 = mybir.dt.bfloat16
BFNP = ml_dtypes.bfloat16

_module_cache: dict = {}


def _build_module(gl: float, ss: float) -> bass.Bass:
    nc = bass.Bass()
    st_in = nc.dram_tensor("st", [T, P, KT // 2, NLOC], BF16, kind="ExternalInput")
    h0_in = nc.dram_tensor("h0", [NLOC, D], F32, kind="ExternalInput")
    h0t_in = nc.dram_tensor("h0t", [LANES, P, MT, P], BF16, kind="ExternalInput")
    thr_in = nc.dram_tensor("thr", [P, LANES, T], F32, kind="ExternalInput")
    r_in = nc.dram_tensor("r", [P, LANES, T], F32, kind="ExternalInput")
    w1_in = nc.dram_tensor("w1", [P, KT, D], BF16, kind="ExternalInput")
    states_o = nc.dram_tensor("states", [T, NLOC, D], F32, kind="ExternalOutput")
    diffs_o = nc.dram_tensor("diffs", [T - 1, NLOC, D], F32, kind="ExternalOutput")
    hfin_o = nc.dram_tensor("hfin", [NLOC, D], F32, kind="ExternalOutput")

    AL = mybir.AluOpType
    AF = mybir.ActivationFunctionType

    with tile.TileContext(nc) as tc, ExitStack() as ctx:
        const = ctx.enter_context(tc.tile_pool(name="const", bufs=1))
        spool = ctx.enter_context(tc.tile_pool(name="spool", bufs=3))
        hpool = ctx.enter_context(tc.tile_pool(name="hpool", bufs=12))
        htpool = ctx.enter_context(tc.tile_pool(name="htpool", bufs=8))
        gdpool = ctx.enter_context(tc.tile_pool(name="gdpool", bufs=12))
        wpool = ctx.enter_context(tc.tile_pool(name="wpool", bufs=8))
        jpool = ctx.enter_context(tc.tile_pool(name="jpool", bufs=4))
        tiny = ctx.enter_context(tc.tile_pool(name="tiny", bufs=8))
        zpool = ctx.enter_context(tc.tile_pool(name="zpool", bufs=4, space="PSUM"))
        tpool = ctx.enter_context(tc.tile_pool(name="tpool", bufs=4, space="PSUM"))

        # ---- constants / initial state ----
        w1_sb = const.tile([P, KT, D], BF16, tag="w1")
        nc.sync.dma_start(out=w1_sb[:], in_=w1_in[:])
        thr_sb = const.tile([P, LANES, T], F32, tag="thr")
        nc.sync.dma_start(out=thr_sb[:], in_=thr_in[:])
        r_sb = const.tile([P, LANES, T], F32, tag="r")
        nc.sync.dma_start(out=r_sb[:], in_=r_in[:])
        ident = const.tile([P, P], F32, tag="ident")
        from concourse.masks import make_identity

        make_identity(nc, ident[:])
        # const_aps isn't wired up in this container: activation() with a
        # float bias on non-Copy funcs asserts. Materialize bias tiles.
        bias0 = const.tile([P, 1], F32, tag="bias0")
        nc.vector.memset(bias0[:], 0.0)
        biasg = const.tile([P, 1], F32, tag="biasg")
        nc.vector.memset(biasg[:], -gl)

        h_cur = []
        ht_cur = []
        for lane in range(LANES):
            h0 = hpool.tile([P, D], F32, tag="h")
            nc.sync.dma_start(out=h0[:], in_=h0_in[lane * P : (lane + 1) * P, :])
            h_cur.append(h0)
            ht0 = htpool.tile([P, MT, P], BF16, tag="ht")
            nc.sync.dma_start(out=ht0[:], in_=h0t_in[lane])
            ht_cur.append(ht0)

        st_cur = spool.tile([P, KT // 2, NLOC], BF16, tag="st")
        nc.sync.dma_start(out=st_cur[:], in_=st_in[0])

        for t in range(T):
            st_next = None
            if t + 1 < T:
                st_next = spool.tile([P, KT // 2, NLOC], BF16, tag="st")
                nc.sync.dma_start(out=st_next[:], in_=st_in[t + 1])

            for lane in range(LANES):
                h = h_cur[lane]
                ht = ht_cur[lane]
                nsl = slice(lane * P, (lane + 1) * P)
                thr_ap = thr_sb[:, lane, t : t + 1]
                r_ap = r_sb[:, lane, t : t + 1]

                # matmul: z = [h | s_t] @ w1 (bf16, fp32 accumulate)
                z = zpool.tile([P, D], F32, tag="z")
                for kt in range(KT // 2):
                    nc.tensor.matmul(
                        out=z[:],
                        lhsT=ht[:, kt, :],
                        rhs=w1_sb[:, kt, :],
                        start=(kt == 0),
                        stop=False,
                    )
                for kt in range(KT // 2):
                    nc.tensor.matmul(
                        out=z[:],
                        lhsT=st_cur[:, kt, nsl],
                        rhs=w1_sb[:, KT // 2 + kt, :],
                        start=False,
                        stop=(kt == KT // 2 - 1),
                    )

                # pre' = z + r*h  (sigmoid applies scale=thr later)
                pre = wpool.tile([P, D], F32, tag="pre")
                nc.vector.scalar_tensor_tensor(
                    out=pre[:], in0=h[:], scalar=r_ap, in1=z[:],
                    op0=AL.mult, op1=AL.add,
                )
                # sig = sigmoid(thr * pre')
                sig = wpool.tile([P, D], F32, tag="sig")
                nc.scalar.activation(
                    out=sig[:], in_=pre[:], func=AF.Sigmoid, scale=thr_ap,
                    bias=bias0[:],
                )
                # d = cand - h = EXPH*sig - h
                d = wpool.tile([P, D], F32, tag="d")
                nc.vector.scalar_tensor_tensor(
                    out=d[:], in0=sig[:], scalar=EXPH, in1=h[:],
                    op0=AL.mult, op1=AL.subtract,
                )
                # sumabs = sum(|d|)
                sumabs = tiny.tile([P, 1], F32, tag="sumabs")
                if lane < 2:
                    junk = jpool.tile([P, D], F32, tag="junk")
                    nc.scalar.activation(
                        out=junk[:], in_=d[:], func=AF.Abs, bias=bias0[:],
                        accum_out=sumabs[:],
                    )
                else:
                    nc.vector.tensor_reduce(
                        out=sumabs[:], in_=d[:], axis=mybir.AxisListType.X,
                        op=AL.add, apply_absolute_value=True,
                    )
                # up = 1-g = sigmoid(-gl - ss*mean|d|)
                up = tiny.tile([P, 1], F32, tag="up")
                nc.scalar.activation(
                    out=up[:], in_=sumabs[:], func=AF.Sigmoid,
                    scale=-ss / D, bias=biasg[:],
                )
                # gd = up * d  (= h_new - h = diffs[t-1])
                gd = gdpool.tile([P, D], F32, tag="gd")
                nc.vector.tensor_scalar(
                    out=gd[:], in0=d[:], scalar1=up[:], scalar2=None, op0=AL.mult
                )
                # h_new = h + gd
                h_new = hpool.tile([P, D], F32, tag="h")
                nc.gpsimd.tensor_tensor(
                    out=h_new[:], in0=gd[:], in1=h[:], op=AL.add
                )

                # transpose h_new for next step's matmul (fp32 PE transpose,
                # cast to bf16 in the PSUM->SBUF copy)
                if t + 1 < T:
                    tps = tpool.tile([P, MT, P], F32, tag="tp")
                    for mt in range(MT):
                        nc.tensor.transpose(
                            out=tps[:, mt, :],
                            in_=h_new[:, mt * P : (mt + 1) * P],
                            identity=ident[:],
                        )
                    ht_new = htpool.tile([P, MT, P], BF16, tag="ht")
                    if lane < 2:
                        nc.scalar.copy(out=ht_new[:], in_=tps[:])
                    else:
                        nc.vector.tensor_copy(out=ht_new[:], in_=tps[:])
                    ht_cur[lane] = ht_new

                # outputs
                nc.sync.dma_start(out=states_o[t, nsl, :], in_=h_new[:])
                if t > 0:
                    nc.sync.dma_start(out=diffs_o[t - 1, nsl, :], in_=gd[:])
                if t == T - 1:
                    nc.sync.dma_start(out=hfin_o[nsl, :], in_=h_new[:])

                h_cur[lane] = h_new

            if st_next is not None:
                st_cur = st_next

    split_excess_syncs(nc)
    return nc


def _get_module(gl: float, ss: float) -> bass.Bass:
    key = (round(gl, 9), round(ss, 9))
    if key not in _module_cache:
        _module_cache[key] = _build_module(gl, ss)
    return _module_cache[key]


def kernel(
    all_data_static,
    threshold_nc,
    all_data_dynamic_now,
    w1,
    smooth_gate_logit,
    smooth_scale,
):
    from concourse.bass_utils import run_bass_kernel_spmd

    gl = float(np.asarray(smooth_gate_logit).reshape(-1)[0])
    ss = float(np.asarray(smooth_scale).reshape(-1)[0])
    nc = _get_module(gl, ss)

    s = np.ascontiguousarray(np.asarray(all_data_static, dtype=np.float32))
    thr_full = np.asarray(threshold_nc, dtype=np.float32).reshape(T, N)
    h0_full = np.asarray(all_data_dynamic_now, dtype=np.float32)
    w1_np = np.asarray(w1, dtype=np.float32)

    # [T, N, D] -> [T, P(k within k-tile), KT/2, N]
    s_prep = np.ascontiguousarray(
        s.reshape(T, N, KT // 2, P).transpose(0, 3, 2, 1)
    ).astype(BFNP)
    w1_prep = np.ascontiguousarray(
        w1_np.reshape(KT, P, D).transpose(1, 0, 2)
    ).astype(BFNP)

    thr_eff = np.maximum(thr_full, 1e-12)
    r_full = (1.0 - thr_full) / thr_eff  # [T, N]

    in_maps = []
    for c in range(NCORES):
        nsl = slice(c * NLOC, (c + 1) * NLOC)
        thr_c = thr_eff[:, nsl].reshape(T, LANES, P).transpose(2, 1, 0)
        r_c = r_full[:, nsl].reshape(T, LANES, P).transpose(2, 1, 0)
        h0_c = h0_full[nsl]  # [NLOC, D]
        h0t_c = (
            h0_c.reshape(LANES, P, MT, P).transpose(0, 3, 2, 1).astype(BFNP)
        )
        in_maps.append(
            {
                "st": np.ascontiguousarray(s_prep[:, :, :, nsl]),
                "h0": np.ascontiguousarray(h0_c),
                "h0t": np.ascontiguousarray(h0t_c),
                "thr": np.ascontiguousarray(thr_c),
                "r": np.ascontiguousarray(r_c),
                "w1": w1_prep,
            }
        )

    res = run_bass_kernel_spmd(nc, in_maps, core_ids=list(range(NCORES)))

    states = np.empty((T, N, D), np.float32)
    diffs = np.empty((T - 1, N, D), np.float32)
    h_final = np.empty((N, D), np.float32)
    for c, out in enumerate(res.results):
        nsl = slice(c * NLOC, (c + 1) * NLOC)
        states[:, nsl, :] = out["states"]
        diffs[:, nsl, :] = out["diffs"]
        h_final[nsl, :] = out["hfin"]
    return states, h_final, diffs


# revision 7
# speedup vs baseline: 1.1199x; 1.1199x over previous
"""Trainium2 Bass kernel for nn_Evolution (gated sigmoid recurrence).

Math per step t (reference):
    z    = concat([h, s_t], -1) @ w1                  # [N, D]
    pre  = z * thr_t + h * (1 - thr_t)
    cand = sigmoid(pre) * exp(-1/2)
    delta = mean(|cand - h|, -1, keepdims=True)
    g    = sigmoid(gl + ss * delta)
    h    = g * h + (1 - g) * cand                     # = h + (1-g)*(cand-h)

Outputs: states [T,N,D] (h after each step), h_final [N,D], diffs
states[1:]-states[:-1] [T-1,N,D].

Device mapping (8 cores, N=4096 sharded 512 rows/core; 4 independent
128-row lanes per core):
  - matmul in bf16: z_lane = sum_k lhsT(k-tile).T @ w1(k-tile) with
    lhsT k-tiles 0-3 = transposed h (maintained on device), 4-7 = host
    pre-transposed s_t. Accumulate fp32 in PSUM. Errors are crushed by
    the thr*z saturation structure (z ~ 110..380, sigmoid 97% saturated;
    non-saturated rows have thr ~ 1/z so err_pre ~ err_z/z ~ 1e-5).
  - pre is computed as thr*(z + r*h) with r = (1-thr)/thr folded into a
    per-partition scalar (host-precomputed), which lets one fused
    scalar_tensor_tensor produce (z + r*h) and the activation's scale
    input apply thr while evaluating sigmoid.
  - 1-g = sigmoid(-gl - ss*delta) directly (per-partition scalar).
  - diffs come for free: diffs[t-1] = h_new - h = (1-g)*(cand-h) = gd.
  - h is re-transposed each step via 4 PE transposes (fp32) and cast to
    bf16 in the PSUM->SBUF copy.
"""

import math
from contextlib import ExitStack

import ml_dtypes
import numpy as np

import concourse.bass as bass
import concourse.mybir as mybir
import concourse.tile as tile


def split_excess_syncs(nc) -> int:
    """Split >1 sync waits/updates per instruction onto adjacent NoOps.

    The walrus in this environment lowers at most one sync wait command
    per instruction (CTRL formats); Tile's kernel-tail drain can carry
    more, failing codegen with "Too many sync wait commands". Moving the
    overflow onto NoOps just before (waits) / after (updates) on the
    same engine is semantically equivalent: per-engine programs execute
    in order.
    """
    MAXW = 1
    MAXU = 1
    nfix = 0
    for f in nc.m.functions:
        for bb in f.blocks:
            snapshot = list(bb.instructions)
            inserts = []  # (index, new_inst) — insert BEFORE index
            for idx, inst in enumerate(snapshot):
                si = inst.sync_info
                if si is None:
                    continue
                waits = list(si.on_wait)
                updates = list(si.on_update)
                if len(waits) <= MAXW and len(updates) <= MAXU:
                    continue
                pre = []
                rest_w = list(waits)
                while len(rest_w) > MAXW:
                    chunk, rest_w = rest_w[:MAXW], rest_w[MAXW:]
                    n = mybir.InstNoOp(
                        name=f"{inst.name}-wsp{len(pre)}", engine=inst.engine
                    )
                    n.sync_info = mybir.SyncInfo(on_wait=chunk, on_update=[])
                    pre.append(n)
                keep_u, rest_u = updates[:MAXU], updates[MAXU:]
                post = []
                while rest_u:
                    chunk, rest_u = rest_u[:MAXU], rest_u[MAXU:]
                    n = mybir.InstNoOp(
                        name=f"{inst.name}-usp{len(post)}", engine=inst.engine
                    )
                    n.sync_info = mybir.SyncInfo(on_wait=[], on_update=chunk)
                    post.append(n)
                inst.sync_info = mybir.SyncInfo(on_wait=rest_w, on_update=keep_u)
                for n in pre:
                    inserts.append((idx, n))
                for n in post:
                    inserts.append((idx + 1, n))
                nfix += 1
            live = bb.instructions
            for index, n in sorted(inserts, key=lambda kv: kv[0], reverse=True):
                live.insert(index, n)
    return nfix

T, N, D = 32, 4096, 512
NCORES = 8
NLOC = N // NCORES  # 512 rows per core
P = 128
LANES = NLOC // P  # 4
KT = (2 * D) // P  # 8 contraction k-tiles
MT = D // P  # 4 m-tiles (columns of h)
EXPH = math.exp(-0.5)

F32 = mybir.dt.float32
BF16 = mybir.dt.bfloat16
BFNP = ml_dtypes.bfloat16

_module_cache: dict = {}


def _build_module(gl: float, ss: float) -> bass.Bass:
    nc = bass.Bass()
    st_in = nc.dram_tensor("st", [T, P, KT // 2, NLOC], BF16, kind="ExternalInput")
    h0_in = nc.dram_tensor("h0", [NLOC, D], F32, kind="ExternalInput")
    h0t_in = nc.dram_tensor("h0t", [LANES, P, MT, P], BF16, kind="ExternalInput")
    thr_in = nc.dram_tensor("thr", [P, LANES, T], F32, kind="ExternalInput")
    r_in = nc.dram_tensor("r", [P, LANES, T], F32, kind="ExternalInput")
    w1_in = nc.dram_tensor("w1", [P, KT, D], BF16, kind="ExternalInput")
    states_o = nc.dram_tensor("states", [T, NLOC, D], F32, kind="ExternalOutput")
    diffs_o = nc.dram_tensor("diffs", [T - 1, NLOC, D], F32, kind="ExternalOutput")
    hfin_o = nc.dram_tensor("hfin", [NLOC, D], F32, kind="ExternalOutput")

    AL = mybir.AluOpType
    AF = mybir.ActivationFunctionType

    with tile.TileContext(nc) as tc, ExitStack() as ctx:
        const = ctx.enter_context(tc.tile_pool(name="const", bufs=1))
        spool = ctx.enter_context(tc.tile_pool(name="spool", bufs=3))
        hpool = ctx.enter_context(tc.tile_pool(name="hpool", bufs=12))
        htpool = ctx.enter_context(tc.tile_pool(name="htpool", bufs=8))
        gdpool = ctx.enter_context(tc.tile_pool(name="gdpool", bufs=12))
        wpool = ctx.enter_context(tc.tile_pool(name="wpool", bufs=8))
        jpool = ctx.enter_context(tc.tile_pool(name="jpool", bufs=4))
        tiny = ctx.enter_context(tc.tile_pool(name="tiny", bufs=8))
        zpool = ctx.enter_context(tc.tile_pool(name="zpool", bufs=4, space="PSUM"))
        tpool = ctx.enter_context(tc.tile_pool(name="tpool", bufs=4, space="PSUM"))

        # ---- constants / initial state ----
        w1_sb = const.tile([P, KT, D], BF16, tag="w1")
        nc.sync.dma_start(out=w1_sb[:], in_=w1_in[:])
        thr_sb = const.tile([P, LANES, T], F32, tag="thr")
        nc.sync.dma_start(out=thr_sb[:], in_=thr_in[:])
        r_sb = const.tile([P, LANES, T], F32, tag="r")
        nc.sync.dma_start(out=r_sb[:], in_=r_in[:])
        ident = const.tile([P, P], F32, tag="ident")
        from concourse.masks import make_identity

        make_identity(nc, ident[:])
        # const_aps isn't wired up in this container: activation() with a
        # float bias on non-Copy funcs asserts. Materialize bias tiles.
        bias0 = const.tile([P, 1], F32, tag="bias0")
        nc.vector.memset(bias0[:], 0.0)
        biasg = const.tile([P, 1], F32, tag="biasg")
        nc.vector.memset(biasg[:], -gl)

        h_cur = []
        ht_cur = []
        for lane in range(LANES):
            h0 = hpool.tile([P, D], F32, tag="h")
            nc.sync.dma_start(out=h0[:], in_=h0_in[lane * P : (lane + 1) * P, :])
            h_cur.append(h0)
            ht0 = htpool.tile([P, MT, P], BF16, tag="ht")
            nc.sync.dma_start(out=ht0[:], in_=h0t_in[lane])
            ht_cur.append(ht0)

        st_cur = spool.tile([P, KT // 2, NLOC], BF16, tag="st")
        nc.sync.dma_start(out=st_cur[:], in_=st_in[0])

        for t in range(T):
            st_next = None
            if t + 1 < T:
                st_next = spool.tile([P, KT // 2, NLOC], BF16, tag="st")
                nc.sync.dma_start(out=st_next[:], in_=st_in[t + 1])

            for lane in range(LANES):
                h = h_cur[lane]
                ht = ht_cur[lane]
                nsl = slice(lane * P, (lane + 1) * P)
                thr_ap = thr_sb[:, lane, t : t + 1]
                r_ap = r_sb[:, lane, t : t + 1]

                # matmul: z = [h | s_t] @ w1 (bf16, fp32 accumulate)
                z = zpool.tile([P, D], F32, tag="z")
                for kt in range(KT // 2):
                    nc.tensor.matmul(
                        out=z[:],
                        lhsT=ht[:, kt, :],
                        rhs=w1_sb[:, kt, :],
                        start=(kt == 0),
                        stop=False,
                    )
                for kt in range(KT // 2):
                    nc.tensor.matmul(
                        out=z[:],
                        lhsT=st_cur[:, kt, nsl],
                        rhs=w1_sb[:, KT // 2 + kt, :],
                        start=False,
                        stop=(kt == KT // 2 - 1),
                    )

                # pre' = z + r*h  (sigmoid applies scale=thr later)
                pre = wpool.tile([P, D], F32, tag="pre")
                nc.vector.scalar_tensor_tensor(
                    out=pre[:], in0=h[:], scalar=r_ap, in1=z[:],
                    op0=AL.mult, op1=AL.add,
                )
                # sig = sigmoid(thr * pre')
                sig = wpool.tile([P, D], F32, tag="sig")
                nc.scalar.activation(
                    out=sig[:], in_=pre[:], func=AF.Sigmoid, scale=thr_ap,
                    bias=bias0[:],
                )
                # d = cand - h = EXPH*sig - h
                d = wpool.tile([P, D], F32, tag="d")
                nc.vector.scalar_tensor_tensor(
                    out=d[:], in0=sig[:], scalar=EXPH, in1=h[:],
                    op0=AL.mult, op1=AL.subtract,
                )
                # sumabs = sum(|d|)
                sumabs = tiny.tile([P, 1], F32, tag="sumabs")
                if lane < 2:
                    junk = jpool.tile([P, D], F32, tag="junk")
                    nc.scalar.activation(
                        out=junk[:], in_=d[:], func=AF.Abs, bias=bias0[:],
                        accum_out=sumabs[:],
                    )
                else:
                    nc.vector.tensor_reduce(
                        out=sumabs[:], in_=d[:], axis=mybir.AxisListType.X,
                        op=AL.add, apply_absolute_value=True,
                    )
                # up = 1-g = sigmoid(-gl - ss*mean|d|)
                up = tiny.tile([P, 1], F32, tag="up")
                nc.scalar.activation(
                    out=up[:], in_=sumabs[:], func=AF.Sigmoid,
                    scale=-ss / D, bias=biasg[:],
                )
                # gd = up * d  (= h_new - h = diffs[t-1]) — ACT Copy w/ scale
                gd = gdpool.tile([P, D], F32, tag="gd")
                nc.scalar.activation(
                    out=gd[:], in_=d[:], func=AF.Copy, scale=up[:]
                )
                # h_new = h + gd
                h_new = hpool.tile([P, D], F32, tag="h")
                nc.gpsimd.tensor_tensor(
                    out=h_new[:], in0=gd[:], in1=h[:], op=AL.add
                )

                # transpose h_new for next step's matmul (fp32 PE transpose,
                # cast to bf16 in the PSUM->SBUF copy)
                if t + 1 < T:
                    tps = tpool.tile([P, MT, P], F32, tag="tp")
                    for mt in range(MT):
                        nc.tensor.transpose(
                            out=tps[:, mt, :],
                            in_=h_new[:, mt * P : (mt + 1) * P],
                            identity=ident[:],
                        )
                    ht_new = htpool.tile([P, MT, P], BF16, tag="ht")
                    if lane < 2:
                        nc.scalar.copy(out=ht_new[:], in_=tps[:])
                    else:
                        nc.vector.tensor_copy(out=ht_new[:], in_=tps[:])
                    ht_cur[lane] = ht_new

                # outputs
                nc.sync.dma_start(out=states_o[t, nsl, :], in_=h_new[:])
                if t > 0:
                    nc.sync.dma_start(out=diffs_o[t - 1, nsl, :], in_=gd[:])
                if t == T - 1:
                    nc.sync.dma_start(out=hfin_o[nsl, :], in_=h_new[:])

                h_cur[lane] = h_new

            if st_next is not None:
                st_cur = st_next

    split_excess_syncs(nc)
    return nc


def _get_module(gl: float, ss: float) -> bass.Bass:
    key = (round(gl, 9), round(ss, 9))
    if key not in _module_cache:
        _module_cache[key] = _build_module(gl, ss)
    return _module_cache[key]


def kernel(
    all_data_static,
    threshold_nc,
    all_data_dynamic_now,
    w1,
    smooth_gate_logit,
    smooth_scale,
):
    from concourse.bass_utils import run_bass_kernel_spmd

    gl = float(np.asarray(smooth_gate_logit).reshape(-1)[0])
    ss = float(np.asarray(smooth_scale).reshape(-1)[0])
    nc = _get_module(gl, ss)

    s = np.ascontiguousarray(np.asarray(all_data_static, dtype=np.float32))
    thr_full = np.asarray(threshold_nc, dtype=np.float32).reshape(T, N)
    h0_full = np.asarray(all_data_dynamic_now, dtype=np.float32)
    w1_np = np.asarray(w1, dtype=np.float32)

    # [T, N, D] -> [T, P(k within k-tile), KT/2, N]
    s_prep = np.ascontiguousarray(
        s.reshape(T, N, KT // 2, P).transpose(0, 3, 2, 1)
    ).astype(BFNP)
    w1_prep = np.ascontiguousarray(
        w1_np.reshape(KT, P, D).transpose(1, 0, 2)
    ).astype(BFNP)

    thr_eff = np.maximum(thr_full, 1e-12)
    r_full = (1.0 - thr_full) / thr_eff  # [T, N]

    in_maps = []
    for c in range(NCORES):
        nsl = slice(c * NLOC, (c + 1) * NLOC)
        thr_c = thr_eff[:, nsl].reshape(T, LANES, P).transpose(2, 1, 0)
        r_c = r_full[:, nsl].reshape(T, LANES, P).transpose(2, 1, 0)
        h0_c = h0_full[nsl]  # [NLOC, D]
        h0t_c = (
            h0_c.reshape(LANES, P, MT, P).transpose(0, 3, 2, 1).astype(BFNP)
        )
        in_maps.append(
            {
                "st": np.ascontiguousarray(s_prep[:, :, :, nsl]),
                "h0": np.ascontiguousarray(h0_c),
                "h0t": np.ascontiguousarray(h0t_c),
                "thr": np.ascontiguousarray(thr_c),
                "r": np.ascontiguousarray(r_c),
                "w1": w1_prep,
            }
        )

    res = run_bass_kernel_spmd(nc, in_maps, core_ids=list(range(NCORES)))

    states = np.empty((T, N, D), np.float32)
    diffs = np.empty((T - 1, N, D), np.float32)
    h_final = np.empty((N, D), np.float32)
    for c, out in enumerate(res.results):
        nsl = slice(c * NLOC, (c + 1) * NLOC)
        states[:, nsl, :] = out["states"]
        diffs[:, nsl, :] = out["diffs"]
        h_final[nsl, :] = out["hfin"]
    return states, h_final, diffs


# revision 14
# speedup vs baseline: 1.2904x; 1.1523x over previous
"""Trainium2 Bass kernel for nn_Evolution (gated sigmoid recurrence).

Math per step t (reference):
    z    = concat([h, s_t], -1) @ w1                  # [N, D]
    pre  = z * thr_t + h * (1 - thr_t)
    cand = sigmoid(pre) * exp(-1/2)
    delta = mean(|cand - h|, -1, keepdims=True)
    g    = sigmoid(gl + ss * delta)
    h    = g * h + (1 - g) * cand                     # = h + (1-g)*(cand-h)

Outputs: states [T,N,D] (h after each step), h_final [N,D], diffs
states[1:]-states[:-1] [T-1,N,D].

Device mapping (8 cores, N=4096 sharded 512 rows/core; 4 independent
128-row lanes per core):
  - matmul in bf16: z_lane = sum_k lhsT(k-tile).T @ w1(k-tile) with
    lhsT k-tiles 0-3 = transposed h (maintained on device), 4-7 = host
    pre-transposed s_t. Accumulate fp32 in PSUM. Errors are crushed by
    the thr*z saturation structure (z ~ 110..380, sigmoid 97% saturated;
    non-saturated rows have thr ~ 1/z so err_pre ~ err_z/z ~ 1e-5).
  - pre is computed as thr*(z + r*h) with r = (1-thr)/thr folded into a
    per-partition scalar (host-precomputed), which lets one fused
    scalar_tensor_tensor produce (z + r*h) and the activation's scale
    input apply thr while evaluating sigmoid.
  - 1-g = sigmoid(-gl - ss*delta) directly (per-partition scalar).
  - diffs come for free: diffs[t-1] = h_new - h = (1-g)*(cand-h) = gd.
  - h is re-transposed each step via 4 PE transposes (fp32) and cast to
    bf16 in the PSUM->SBUF copy.
"""

import math
from contextlib import ExitStack

import ml_dtypes
import numpy as np

import concourse.bass as bass
import concourse.mybir as mybir
import concourse.tile as tile


def split_excess_syncs(nc) -> int:
    """Split >1 sync waits/updates per instruction onto adjacent NoOps.

    The walrus in this environment lowers at most one sync wait command
    per instruction (CTRL formats); Tile's kernel-tail drain can carry
    more, failing codegen with "Too many sync wait commands". Moving the
    overflow onto NoOps just before (waits) / after (updates) on the
    same engine is semantically equivalent: per-engine programs execute
    in order.
    """
    MAXW = 1
    MAXU = 1
    nfix = 0
    for f in nc.m.functions:
        for bb in f.blocks:
            snapshot = list(bb.instructions)
            inserts = []  # (index, new_inst) — insert BEFORE index
            for idx, inst in enumerate(snapshot):
                si = inst.sync_info
                if si is None:
                    continue
                waits = list(si.on_wait)
                updates = list(si.on_update)
                if len(waits) <= MAXW and len(updates) <= MAXU:
                    continue
                pre = []
                rest_w = list(waits)
                while len(rest_w) > MAXW:
                    chunk, rest_w = rest_w[:MAXW], rest_w[MAXW:]
                    n = mybir.InstNoOp(
                        name=f"{inst.name}-wsp{len(pre)}", engine=inst.engine
                    )
                    n.sync_info = mybir.SyncInfo(on_wait=chunk, on_update=[])
                    pre.append(n)
                keep_u, rest_u = updates[:MAXU], updates[MAXU:]
                post = []
                while rest_u:
                    chunk, rest_u = rest_u[:MAXU], rest_u[MAXU:]
                    n = mybir.InstNoOp(
                        name=f"{inst.name}-usp{len(post)}", engine=inst.engine
                    )
                    n.sync_info = mybir.SyncInfo(on_wait=[], on_update=chunk)
                    post.append(n)
                inst.sync_info = mybir.SyncInfo(on_wait=rest_w, on_update=keep_u)
                for n in pre:
                    inserts.append((idx, n))
                for n in post:
                    inserts.append((idx + 1, n))
                nfix += 1
            live = bb.instructions
            for index, n in sorted(inserts, key=lambda kv: kv[0], reverse=True):
                live.insert(index, n)
    return nfix

T, N, D = 32, 4096, 512
NCORES = 8
NLOC = N // NCORES  # 512 rows per core
P = 128
LANES = NLOC // P  # 4
KT = (2 * D) // P  # 8 contraction k-tiles
MT = D // P  # 4 m-tiles (columns of h)
EXPH = math.exp(-0.5)

F32 = mybir.dt.float32
BF16 = mybir.dt.bfloat16
BFNP = ml_dtypes.bfloat16

_module_cache: dict = {}


def _build_module(gl: float, ss: float) -> bass.Bass:
    nc = bass.Bass()
    st_in = nc.dram_tensor("st", [T, P, KT // 2, NLOC], BF16, kind="ExternalInput")
    h0_in = nc.dram_tensor("h0", [NLOC, D], F32, kind="ExternalInput")
    h0t_in = nc.dram_tensor("h0t", [LANES, P, MT, P], BF16, kind="ExternalInput")
    thr_in = nc.dram_tensor("thr", [P, LANES, T], F32, kind="ExternalInput")
    r_in = nc.dram_tensor("r", [P, LANES, T], F32, kind="ExternalInput")
    w1_in = nc.dram_tensor("w1", [P, KT, D], BF16, kind="ExternalInput")
    states_o = nc.dram_tensor("states", [T, NLOC, D], F32, kind="ExternalOutput")
    diffs_o = nc.dram_tensor("diffs", [T - 1, NLOC, D], F32, kind="ExternalOutput")
    hfin_o = nc.dram_tensor("hfin", [NLOC, D], F32, kind="ExternalOutput")

    AL = mybir.AluOpType
    AF = mybir.ActivationFunctionType

    with tile.TileContext(nc) as tc, ExitStack() as ctx:
        const = ctx.enter_context(tc.tile_pool(name="const", bufs=1))
        spool = ctx.enter_context(tc.tile_pool(name="spool", bufs=3))
        hpool = ctx.enter_context(tc.tile_pool(name="hpool", bufs=12))
        htpool = ctx.enter_context(tc.tile_pool(name="htpool", bufs=8))
        gdpool = ctx.enter_context(tc.tile_pool(name="gdpool", bufs=12))
        wpool = ctx.enter_context(tc.tile_pool(name="wpool", bufs=8))
        jpool = ctx.enter_context(tc.tile_pool(name="jpool", bufs=4))
        tiny = ctx.enter_context(tc.tile_pool(name="tiny", bufs=8))
        zpool = ctx.enter_context(tc.tile_pool(name="zpool", bufs=4, space="PSUM"))
        tpool = ctx.enter_context(tc.tile_pool(name="tpool", bufs=4, space="PSUM"))

        # ---- constants / initial state ----
        w1_sb = const.tile([P, KT, D], BF16, tag="w1")
        nc.sync.dma_start(out=w1_sb[:], in_=w1_in[:])
        thr_sb = const.tile([P, LANES, T], F32, tag="thr")
        nc.sync.dma_start(out=thr_sb[:], in_=thr_in[:])
        r_sb = const.tile([P, LANES, T], F32, tag="r")
        nc.sync.dma_start(out=r_sb[:], in_=r_in[:])
        ident = const.tile([P, P], F32, tag="ident")
        from concourse.masks import make_identity

        make_identity(nc, ident[:])
        # const_aps isn't wired up in this container: activation() with a
        # float bias on non-Copy funcs asserts. Materialize bias tiles.
        bias0 = const.tile([P, 1], F32, tag="bias0")
        nc.vector.memset(bias0[:], 0.0)
        biasg = const.tile([P, 1], F32, tag="biasg")
        nc.vector.memset(biasg[:], -gl)

        h_cur = []
        ht_cur = []
        for lane in range(LANES):
            h0 = hpool.tile([P, D], F32, tag="h")
            nc.sync.dma_start(out=h0[:], in_=h0_in[lane * P : (lane + 1) * P, :])
            h_cur.append(h0)
            ht0 = htpool.tile([P, MT, P], BF16, tag="ht")
            nc.sync.dma_start(out=ht0[:], in_=h0t_in[lane])
            ht_cur.append(ht0)

        st_cur = spool.tile([P, KT // 2, NLOC], BF16, tag="st")
        nc.sync.dma_start(out=st_cur[:], in_=st_in[0])

        for t in range(T):
            st_next = None
            if t + 1 < T:
                st_next = spool.tile([P, KT // 2, NLOC], BF16, tag="st")
                nc.sync.dma_start(out=st_next[:], in_=st_in[t + 1])

            for lane in range(LANES):
                h = h_cur[lane]
                ht = ht_cur[lane]
                nsl = slice(lane * P, (lane + 1) * P)
                thr_ap = thr_sb[:, lane, t : t + 1]
                r_ap = r_sb[:, lane, t : t + 1]

                # matmul: z = [h | s_t] @ w1 (bf16, fp32 accumulate)
                z = zpool.tile([P, D], F32, tag="z")
                for kt in range(KT // 2):
                    nc.tensor.matmul(
                        out=z[:],
                        lhsT=ht[:, kt, :],
                        rhs=w1_sb[:, kt, :],
                        start=(kt == 0),
                        stop=False,
                    )
                for kt in range(KT // 2):
                    nc.tensor.matmul(
                        out=z[:],
                        lhsT=st_cur[:, kt, nsl],
                        rhs=w1_sb[:, KT // 2 + kt, :],
                        start=False,
                        stop=(kt == KT // 2 - 1),
                    )

                # pre' = z + r*h  (sigmoid applies scale=thr later)
                pre = wpool.tile([P, D], F32, tag="pre")
                nc.vector.scalar_tensor_tensor(
                    out=pre[:], in0=h[:], scalar=r_ap, in1=z[:],
                    op0=AL.mult, op1=AL.add,
                )
                # sig = sigmoid(thr * pre')
                sig = wpool.tile([P, D], F32, tag="sig")
                nc.scalar.activation(
                    out=sig[:], in_=pre[:], func=AF.Sigmoid, scale=thr_ap,
                    bias=bias0[:],
                )
                # d = cand - h = EXPH*sig - h
                d = wpool.tile([P, D], F32, tag="d")
                nc.vector.scalar_tensor_tensor(
                    out=d[:], in0=sig[:], scalar=EXPH, in1=h[:],
                    op0=AL.mult, op1=AL.subtract,
                )
                # sumabs = sum(|d|)
                sumabs = tiny.tile([P, 1], F32, tag="sumabs")
                if lane < 2:
                    junk = jpool.tile([P, D], F32, tag="junk")
                    nc.scalar.activation(
                        out=junk[:], in_=d[:], func=AF.Abs, bias=bias0[:],
                        accum_out=sumabs[:],
                    )
                else:
                    nc.vector.tensor_reduce(
                        out=sumabs[:], in_=d[:], axis=mybir.AxisListType.X,
                        op=AL.add, apply_absolute_value=True,
                    )
                # up = 1-g = sigmoid(-gl - ss*mean|d|)
                up = tiny.tile([P, 1], F32, tag="up")
                nc.scalar.activation(
                    out=up[:], in_=sumabs[:], func=AF.Sigmoid,
                    scale=-ss / D, bias=biasg[:],
                )
                # gd = up * d  (= h_new - h = diffs[t-1]) — ACT Copy w/ scale
                gd = gdpool.tile([P, D], F32, tag="gd")
                nc.scalar.activation(
                    out=gd[:], in_=d[:], func=AF.Copy, scale=up[:]
                )
                # h_new = h + gd
                h_new = hpool.tile([P, D], F32, tag="h")
                nc.gpsimd.tensor_tensor(
                    out=h_new[:], in0=gd[:], in1=h[:], op=AL.add
                )

                # hT update without waiting for h_new: transpose gd and add
                # to the previous hT (bf16 accumulate; drift only feeds the
                # matmul operand, where the thr*z structure crushes it)
                if t + 1 < T:
                    tps = tpool.tile([P, MT, P], F32, tag="tp")
                    for mt in range(MT):
                        nc.tensor.transpose(
                            out=tps[:, mt, :],
                            in_=gd[:, mt * P : (mt + 1) * P],
                            identity=ident[:],
                        )
                    ht_new = htpool.tile([P, MT, P], BF16, tag="ht")
                    nc.vector.tensor_tensor(
                        out=ht_new[:], in0=tps[:], in1=ht[:], op=AL.add
                    )
                    ht_cur[lane] = ht_new

                # outputs
                nc.sync.dma_start(out=states_o[t, nsl, :], in_=h_new[:])
                if t > 0:
                    nc.sync.dma_start(out=diffs_o[t - 1, nsl, :], in_=gd[:])
                if t == T - 1:
                    nc.sync.dma_start(out=hfin_o[nsl, :], in_=h_new[:])

                h_cur[lane] = h_new

            if st_next is not None:
                st_cur = st_next

    split_excess_syncs(nc)
    return nc


def _get_module(gl: float, ss: float) -> bass.Bass:
    key = (round(gl, 9), round(ss, 9))
    if key not in _module_cache:
        _module_cache[key] = _build_module(gl, ss)
    return _module_cache[key]


def kernel(
    all_data_static,
    threshold_nc,
    all_data_dynamic_now,
    w1,
    smooth_gate_logit,
    smooth_scale,
):
    from concourse.bass_utils import run_bass_kernel_spmd

    gl = float(np.asarray(smooth_gate_logit).reshape(-1)[0])
    ss = float(np.asarray(smooth_scale).reshape(-1)[0])
    nc = _get_module(gl, ss)

    s = np.ascontiguousarray(np.asarray(all_data_static, dtype=np.float32))
    thr_full = np.asarray(threshold_nc, dtype=np.float32).reshape(T, N)
    h0_full = np.asarray(all_data_dynamic_now, dtype=np.float32)
    w1_np = np.asarray(w1, dtype=np.float32)

    # [T, N, D] -> [T, P(k within k-tile), KT/2, N]
    s_prep = np.ascontiguousarray(
        s.reshape(T, N, KT // 2, P).transpose(0, 3, 2, 1)
    ).astype(BFNP)
    w1_prep = np.ascontiguousarray(
        w1_np.reshape(KT, P, D).transpose(1, 0, 2)
    ).astype(BFNP)

    thr_eff = np.maximum(thr_full, 1e-12)
    r_full = (1.0 - thr_full) / thr_eff  # [T, N]

    in_maps = []
    for c in range(NCORES):
        nsl = slice(c * NLOC, (c + 1) * NLOC)
        thr_c = thr_eff[:, nsl].reshape(T, LANES, P).transpose(2, 1, 0)
        r_c = r_full[:, nsl].reshape(T, LANES, P).transpose(2, 1, 0)
        h0_c = h0_full[nsl]  # [NLOC, D]
        h0t_c = (
            h0_c.reshape(LANES, P, MT, P).transpose(0, 3, 2, 1).astype(BFNP)
        )
        in_maps.append(
            {
                "st": np.ascontiguousarray(s_prep[:, :, :, nsl]),
                "h0": np.ascontiguousarray(h0_c),
                "h0t": np.ascontiguousarray(h0t_c),
                "thr": np.ascontiguousarray(thr_c),
                "r": np.ascontiguousarray(r_c),
                "w1": w1_prep,
            }
        )

    res = run_bass_kernel_spmd(nc, in_maps, core_ids=list(range(NCORES)))

    states = np.empty((T, N, D), np.float32)
    diffs = np.empty((T - 1, N, D), np.float32)
    h_final = np.empty((N, D), np.float32)
    for c, out in enumerate(res.results):
        nsl = slice(c * NLOC, (c + 1) * NLOC)
        states[:, nsl, :] = out["states"]
        diffs[:, nsl, :] = out["diffs"]
        h_final[nsl, :] = out["hfin"]
    return states, h_final, diffs
